# revision 2
# baseline (speedup 1.0000x reference)
"""Trainium2 Bass kernel for attention with per-head qk-layernorm.

Problem (hardcoded): B=2, N=4096, C=1024, H=16, D=64, f32 I/O.
  qkv = x @ qkv_w.T + qkv_b ; per-head LN(q), LN(k) (eps 1e-5)
  attn = softmax(q*D^-0.5 @ k.T) @ v ; out = attn @ proj_w.T + proj_b

Sharding (8 cores): core c -> batch b=c//4, query rows [1024*(c%4), +1024).
Each core computes q,k,v for its own 1024 rows (all 16 heads), AllGathers
k^T/v across its 4-core batch group, runs flash attention for its query rows
over the full 4096-key sequence, and projects. Output needs no collective:
host concatenates the 8 [1024,1024] slices.

Structure (v1):
  A-kv: k,v for all 8 token tiles first -> AllGather chunks issue early.
        Transposes lag one tile behind the QKV matmuls so the PE never
        waits on the LN vector chain. Head-pair [128,128] transposes halve
        the transpose count. QKV bias is added during the PSUM->SBUF copy
        against a broadcast bias tile (no K=1 bias matmuls).
  A-q:  q for all 8 tiles, same pipeline.
  C:    flash attention per head-pair; exp in [128,1536] activation tiles
        (3 key tiles per instr); key-chunk order rotated per head-pair to
        tolerate AllGather stragglers.
  D:    output projection, bias fused into the PSUM->SBUF copy.

Numerics: matmuls bf16 with f32 PSUM accumulation. Softmax skips
max-subtraction: LN guarantees ||q_row||,||k_row|| <= sqrt(D)=8, so
|S|*scale <= 8 -> exp safe. Softmax denominators come from a ones-column
appended to V (row 64 of the PV accumulator).
"""

import os
import sys

for _p in ("/opt/trn_rl_repo", "/root/.axon_site/_ro/trn_rl_repo"):
    if os.path.isdir(_p) and _p not in sys.path:
        sys.path.insert(0, _p)

import numpy as np
import ml_dtypes

B, N, C = 2, 4096, 1024
H, D = 16, 64
NLOC = N // 4          # query rows per core = 1024
P = 128                # partitions
LN_EPS = 1e-5
SCALE = D ** -0.5
N_CORES = 8
BF16 = ml_dtypes.bfloat16

_COMPILED = {}


def build_graph():
    import concourse.bass as bass
    import concourse.mybir as mybir
    import concourse.tile as tile
    from concourse import bacc
    from concourse.masks import make_identity

    fp32 = mybir.dt.float32
    bf16 = mybir.dt.bfloat16
    AF = mybir.ActivationFunctionType
    ALU = mybir.AluOpType
    AX = mybir.AxisListType

    nc = bacc.Bacc(trn_type="TRN2", target_bir_lowering=False, num_devices=N_CORES)

    # ---- I/O -------------------------------------------------------------
    xT = nc.declare_dram_parameter("xT", [C, NLOC], bf16, isOutput=False)          # x slice, transposed
    wqkvT = nc.declare_dram_parameter("wqkvT", [C, 3 * C], bf16, isOutput=False)   # qkv_w.T
    qkvb = nc.declare_dram_parameter("qkvb", [1, 3 * C], fp32, isOutput=False)
    wpT = nc.declare_dram_parameter("wpT", [C, C], bf16, isOutput=False)           # proj_w.T
    pb = nc.declare_dram_parameter("pb", [1, C], fp32, isOutput=False)
    qn_wb = nc.declare_dram_parameter("qn_wb", [P, 2], fp32, isOutput=False)       # [:,0]=w [:,1]=b, tiled x2
    kn_wb = nc.declare_dram_parameter("kn_wb", [P, 2], fp32, isOutput=False)
    out = nc.declare_dram_parameter("out", [NLOC, C], fp32, isOutput=True)

    NT = NLOC // P        # 8 local row tiles
    HP = H // 2           # 8 head pairs
    KT = N // P           # 32 key tiles
    SL = 2 * (D + 1)      # 130: [vA(64)|1|vB(64)|1] per key tile

    rg = [[0, 1, 2, 3], [4, 5, 6, 7]]

    with tile.TileContext(nc) as tc:
        with (
            tc.tile_pool(name="const", bufs=1) as const,
            tc.tile_pool(name="persist", bufs=1) as persist,
            tc.tile_pool(name="dram", bufs=1, space="DRAM") as dram,
        ):
            # ---------------- preamble constants -------------------------
            ident = const.tile([P, P], bf16, tag="ident", name="ident")
            make_identity(nc, ident)
            eps_t = const.tile([P, 1], fp32, tag="eps_t", name="eps_t")
            nc.any.memset(eps_t[:], LN_EPS)

            qkvb_f = const.tile([1, 3 * C], fp32, tag="qkvb_f", name="qkvb_f")
            nc.sync.dma_start(qkvb_f[:], qkvb[:])
            qkvb_bf = const.tile([1, 3 * C], bf16, tag="qkvb_bf", name="qkvb_bf")
            nc.vector.tensor_copy(qkvb_bf[:], qkvb_f[:])
            qkvb_bc = const.tile([P, 3 * C], bf16, tag="qkvb_bc", name="qkvb_bc")
            nc.gpsimd.partition_broadcast(qkvb_bc[:], qkvb_bf[:], channels=P)

            pb_f = const.tile([1, C], fp32, tag="pb_f", name="pb_f")
            nc.sync.dma_start(pb_f[:], pb[:])
            pb_bc = const.tile([P, C], fp32, tag="pb_bc", name="pb_bc")
            nc.gpsimd.partition_broadcast(pb_bc[:], pb_f[:], channels=P)

            qnwb = const.tile([P, 2], fp32, tag="qnwb", name="qnwb")
            nc.sync.dma_start(qnwb[:], qn_wb[:])
            knwb = const.tile([P, 2], fp32, tag="knwb", name="knwb")
            nc.sync.dma_start(knwb[:], kn_wb[:])

            # persistent SBUF: qT (pair-major) and attnT accumulators
            qT_sb = [persist.tile([P, NLOC], bf16, tag=f"qT{g}", name=f"qT{g}") for g in range(HP)]
            attnT = [persist.tile([P, NLOC], bf16, tag=f"aT{g}", name=f"aT{g}") for g in range(HP)]

            # DRAM kv chunks: rows [0:128]=kT_stage_i, [128:256]=v_i
            kv_loc = [dram.tile([256, C], bf16, tag=f"kvl{i}", name=f"kvl{i}")
                      for i in range(NT)]
            kv_ful = [dram.tile([1024, C], bf16, tag=f"kvf{i}", name=f"kvf{i}")
                      for i in range(NT)]

            # ================= Phase A: QKV + LN + transposes =============
            with (
                tc.tile_pool(name="qkv_ps", bufs=4, space="PSUM") as qkv_ps,
                tc.tile_pool(name="tp_ps", bufs=3, space="PSUM") as tp_ps,
                tc.tile_pool(name="ln", bufs=3) as ln_pool,
                tc.tile_pool(name="kv_stage", bufs=2) as kv_stage,
                tc.tile_pool(name="pa_w", bufs=1) as pa_w,
            ):
                xT_sb = [pa_w.tile([P, NLOC], bf16, tag=f"xT{i}", name=f"xT{i}") for i in range(NT)]
                wq_sb = [pa_w.tile([P, 3 * C], bf16, tag=f"wq{i}", name=f"wq{i}") for i in range(NT)]
                for i in range(NT):
                    nc.sync.dma_start(xT_sb[i][:], xT[i * P:(i + 1) * P, :])
                    nc.sync.dma_start(wq_sb[i][:], wqkvT[i * P:(i + 1) * P, :])

                def qkv_mm(i, j):
                    """x-tile i against qkv-weight chunk j -> PSUM [128,512]."""
                    ps = qkv_ps.tile([P, 512], fp32, tag="ps", name="ps")
                    for kk in range(8):
                        nc.tensor.matmul(
                            ps[:],
                            xT_sb[kk][:, i * P:(i + 1) * P],
                            wq_sb[kk][:, j * 512:(j + 1) * 512],
                            start=(kk == 0), stop=(kk == 7))
                    return ps

                def ln_stats(t_f, pool_tag):
                    """LN stats for [128, C] f32 with 16 heads of 64."""
                    t3 = t_f[:].rearrange("p (h d) -> p h d", d=D)
                    sums = ln_pool.tile([P, H], fp32, tag=f"{pool_tag}sum", name=f"{pool_tag}sum")
                    nc.vector.tensor_reduce(sums[:], t3, axis=AX.X, op=ALU.add)
                    sq = ln_pool.tile([P, C], fp32, tag=f"{pool_tag}sq", name=f"{pool_tag}sq")
                    nc.scalar.square(sq[:], t_f[:])
                    ssq = ln_pool.tile([P, H], fp32, tag=f"{pool_tag}ssq", name=f"{pool_tag}ssq")
                    nc.vector.tensor_reduce(
                        ssq[:], sq[:].rearrange("p (h d) -> p h d", d=D),
                        axis=AX.X, op=ALU.add)
                    mu = ln_pool.tile([P, H], fp32, tag=f"{pool_tag}mu", name=f"{pool_tag}mu")
                    nc.vector.tensor_scalar_mul(mu[:], sums[:], 1.0 / D)
                    mu2 = ln_pool.tile([P, H], fp32, tag=f"{pool_tag}mu2", name=f"{pool_tag}mu2")
                    nc.vector.tensor_mul(mu2[:], mu[:], mu[:])
                    var = ln_pool.tile([P, H], fp32, tag=f"{pool_tag}var", name=f"{pool_tag}var")
                    nc.vector.scalar_tensor_tensor(
                        var[:], ssq[:], 1.0 / D, mu2[:],
                        op0=ALU.mult, op1=ALU.subtract)
                    sig = ln_pool.tile([P, H], fp32, tag=f"{pool_tag}sig", name=f"{pool_tag}sig")
                    nc.scalar.activation(sig[:], var[:], AF.Sqrt, bias=eps_t[:])
                    rstd = ln_pool.tile([P, H], fp32, tag=f"{pool_tag}rstd", name=f"{pool_tag}rstd")
                    nc.vector.reciprocal(rstd[:], sig[:])
                    nmr = ln_pool.tile([P, H], fp32, tag=f"{pool_tag}nmr", name=f"{pool_tag}nmr")
                    nc.vector.scalar_tensor_tensor(
                        nmr[:], mu[:], -1.0, rstd[:], op0=ALU.mult, op1=ALU.mult)
                    return mu, rstd, nmr

                def ln_normalize(t_f, mu, rstd, nmr, pool_tag):
                    """tn = (t - mu) * rstd, split across DVE and Act."""
                    tn = ln_pool.tile([P, C], bf16, tag=f"{pool_tag}n", name=f"{pool_tag}n")
                    for h in range(H):
                        sl = slice(h * D, (h + 1) * D)
                        if h % 2 == 0:
                            nc.vector.tensor_scalar(
                                tn[:, sl], t_f[:, sl],
                                mu[:, h:h + 1], rstd[:, h:h + 1],
                                op0=ALU.subtract, op1=ALU.mult)
                        else:
                            nc.scalar.activation(
                                tn[:, sl], t_f[:, sl], AF.Identity,
                                bias=nmr[:, h:h + 1], scale=rstd[:, h:h + 1])
                    return tn

                # ---- A-kv: k and v for all tiles, transposes lag 1 tile --
                k_work = []  # deferred per-tile transpose emitters

                def emit_kv_tile(i):
                    k_f = ln_pool.tile([P, C], fp32, tag="k_f", name="k_f")
                    for j in (2, 3):
                        ps = qkv_mm(i, j)
                        nc.vector.tensor_tensor(
                            k_f[:, (j - 2) * 512:(j - 1) * 512], ps[:],
                            qkvb_bc[:, j * 512:(j + 1) * 512], op=ALU.add)
                    v_bf = kv_stage.tile([P, C], bf16, tag="v_bf", name="v_bf")
                    for j in (4, 5):
                        ps = qkv_mm(i, j)
                        nc.vector.tensor_tensor(
                            v_bf[:, (j - 4) * 512:(j - 3) * 512], ps[:],
                            qkvb_bc[:, j * 512:(j + 1) * 512], op=ALU.add)
                    nc.sync.dma_start(kv_loc[i][P:2 * P, :], v_bf[:])
                    mu, rstd, nmr = ln_stats(k_f, "k")
                    tn = ln_normalize(k_f, mu, rstd, nmr, "k")

                    def do_transposes():
                        kT_stage = kv_stage.tile([P, C], bf16, tag="kT_stage", name="kT_stage")
                        for g in range(HP):
                            tp = tp_ps.tile([P, P], bf16, tag="tp", name="tp")
                            nc.tensor.transpose(tp[:], tn[:, g * P:(g + 1) * P], ident[:])
                            nc.vector.tensor_scalar(
                                kT_stage[:, g * P:(g + 1) * P],
                                tp[:], knwb[:, 0:1], knwb[:, 1:2],
                                op0=ALU.mult, op1=ALU.add)
                        nc.sync.dma_start(kv_loc[i][0:P, :], kT_stage[:])
                        nc.gpsimd.collective_compute(
                            "AllGather", mybir.AluOpType.bypass,
                            replica_groups=rg,
                            ins=[kv_loc[i][:].opt()],
                            outs=[kv_ful[i][:].opt()])
                    return do_transposes

                for i in range(NT):
                    k_work.append(emit_kv_tile(i))
                    if i >= 1:
                        k_work[i - 1]()
                k_work[NT - 1]()

                # ---- A-q: q for all tiles, transposes lag 1 tile ---------
                q_work = []

                def emit_q_tile(i):
                    q_f = ln_pool.tile([P, C], fp32, tag="q_f", name="q_f")
                    for j in (0, 1):
                        ps = qkv_mm(i, j)
                        nc.vector.tensor_tensor(
                            q_f[:, j * 512:(j + 1) * 512], ps[:],
                            qkvb_bc[:, j * 512:(j + 1) * 512], op=ALU.add)
                    mu, rstd, nmr = ln_stats(q_f, "q")
                    tn = ln_normalize(q_f, mu, rstd, nmr, "q")

                    def do_transposes():
                        for g in range(HP):
                            tp = tp_ps.tile([P, P], bf16, tag="tp", name="tp")
                            nc.tensor.transpose(tp[:], tn[:, g * P:(g + 1) * P], ident[:])
                            nc.vector.tensor_scalar(
                                qT_sb[g][:, i * P:(i + 1) * P],
                                tp[:], qnwb[:, 0:1], qnwb[:, 1:2],
                                op0=ALU.mult, op1=ALU.add)
                    return do_transposes

                for i in range(NT):
                    q_work.append(emit_q_tile(i))
                    if i >= 1:
                        q_work[i - 1]()
                q_work[NT - 1]()

            # ================= Phases C+D ================================
            with (
                tc.tile_pool(name="kv_sb", bufs=2) as kv_sb,
                tc.tile_pool(name="p_sb", bufs=2) as p_sb,
                tc.tile_pool(name="nrm", bufs=2) as nrm,
                tc.tile_pool(name="pd_w", bufs=1) as pd_w,
                tc.tile_pool(name="y_sb", bufs=2) as y_sb_pool,
            ):
                wp_sb = [pd_w.tile([P, C], bf16, tag=f"wp{i}", name=f"wp{i}") for i in range(NT)]
                for i in range(NT):
                    nc.sync.dma_start(wp_sb[i][:], wpT[i * P:(i + 1) * P, :])

                # ---------------- Phase C: flash attention ---------------
                with (
                    tc.tile_pool(name="st_ps", bufs=2, space="PSUM") as st_ps,
                    tc.tile_pool(name="o_ps", bufs=1, space="PSUM") as o_ps,
                ):
                    for hp in range(HP):
                        # stage k^T and v(+ones) for this head pair, in the
                        # rotated chunk order this hp consumes them
                        chunk_order = [(hp + j) % NT for j in range(NT)]
                        kT_i = [None] * NT
                        va_i = [None] * NT
                        for i in chunk_order:
                            kt = kv_sb.tile([P, 4 * P], bf16, tag=f"kT{i}", name=f"kT{i}")
                            nc.sync.dma_start(
                                kt[:].rearrange("p (b n) -> p b n", b=4),
                                kv_ful[i][:, hp * P:(hp + 1) * P].rearrange(
                                    "(b q p) c -> p b q c", q=2, p=P)[:, :, 0, :])
                            kT_i[i] = kt
                            va = kv_sb.tile([P, 4 * SL], bf16, tag=f"va{i}", name=f"va{i}")
                            nc.vector.memset(va[:, D::(D + 1)], 1.0)
                            for hh in range(2):
                                nc.sync.dma_start(
                                    va[:].rearrange("p (b d) -> p b d", d=SL)[
                                        :, :, hh * (D + 1): hh * (D + 1) + D],
                                    kv_ful[i][:, hp * P + hh * D: hp * P + (hh + 1) * D
                                              ].rearrange("(b q p) d -> p b q d",
                                                          q=2, p=P)[:, :, 1, :])
                            va_i[i] = va

                        t_seq = [4 * c + b for c in chunk_order for b in range(4)]
                        blocks = [t_seq[o:o + 3] for o in range(0, KT, 3)]  # 10x3 + 1x2

                        for m in range(2):
                            o_t = [o_ps.tile([D + 1, 512], fp32, tag=f"o{hh}", name=f"o{hh}")
                                   for hh in range(2)]
                            pv_pos = [0, 0]
                            pend = []

                            def issue_pv(p_t, blk, hh):
                                for u, t in enumerate(blk):
                                    i, b = t // 4, t % 4
                                    nc.tensor.matmul(
                                        o_t[hh][:],
                                        va_i[i][:, b * SL + hh * (D + 1):
                                                b * SL + (hh + 1) * (D + 1)],
                                        p_t[:, u * 512:(u + 1) * 512],
                                        start=(pv_pos[hh] == 0),
                                        stop=(pv_pos[hh] == KT - 1))
                                    pv_pos[hh] += 1

                            for blk in blocks:
                                w = len(blk) * 512
                                for hh in range(2):
                                    st = st_ps.tile([P, 1536], fp32, tag="st", name="st")
                                    for u, t in enumerate(blk):
                                        i, b = t // 4, t % 4
                                        nc.tensor.matmul(
                                            st[:, u * 512:(u + 1) * 512],
                                            kT_i[i][hh * D:(hh + 1) * D, b * P:(b + 1) * P],
                                            qT_sb[hp][hh * D:(hh + 1) * D,
                                                      m * 512:(m + 1) * 512],
                                            start=True, stop=True)
                                    p_t = p_sb.tile([P, 1536], bf16, tag=f"p{hh}",
                                                    name=f"p{hh}")
                                    nc.scalar.activation(p_t[:, :w], st[:, :w],
                                                         AF.Exp, scale=SCALE)
                                    pend.append((p_t, blk, hh))
                                while len(pend) > 2:
                                    issue_pv(*pend.pop(0))
                            for args in pend:
                                issue_pv(*args)

                            for hh in range(2):
                                linv = nrm.tile([1, 512], fp32, tag=f"li{hh}", name=f"li{hh}")
                                nc.vector.reciprocal(linv[:], o_t[hh][D:D + 1, :])
                                bc_sb = nrm.tile([D, 512], fp32, tag=f"bs{hh}", name=f"bs{hh}")
                                nc.gpsimd.partition_broadcast(bc_sb[:], linv[:], channels=D)
                                nc.vector.tensor_mul(
                                    attnT[hp][hh * D:(hh + 1) * D, m * 512:(m + 1) * 512],
                                    o_t[hh][0:D, :], bc_sb[:])

                # ---------------- Phase D: output projection -------------
                with tc.tile_pool(name="y_ps", bufs=2, space="PSUM") as y_ps:
                    for i in range(NT):
                        y_t = y_sb_pool.tile([P, C], fp32, tag="y", name="y")
                        for co in range(2):
                            yp = y_ps.tile([P, 512], fp32, tag="yp", name="yp")
                            for g in range(HP):
                                nc.tensor.matmul(
                                    yp[:],
                                    attnT[g][:, i * P:(i + 1) * P],
                                    wp_sb[g][:, co * 512:(co + 1) * 512],
                                    start=(g == 0), stop=(g == 7))
                            nc.vector.tensor_tensor(
                                y_t[:, co * 512:(co + 1) * 512], yp[:],
                                pb_bc[:, co * 512:(co + 1) * 512], op=ALU.add)
                        nc.sync.dma_start(out[i * P:(i + 1) * P, :], y_t[:])

    nc.finalize()
    return nc


def _prep_in_maps(x, qkv_w, qkv_b, q_norm_w, q_norm_b, k_norm_w, k_norm_b,
                  proj_w, proj_b):
    wqkvT = np.ascontiguousarray(qkv_w.T).astype(BF16)
    wpT = np.ascontiguousarray(proj_w.T).astype(BF16)
    qkvb = qkv_b.reshape(1, 3 * C).astype(np.float32)
    pb = proj_b.reshape(1, C).astype(np.float32)
    # LN affine applied post-transpose on [128 = (head-of-pair, d)] partitions
    qn_wb = np.stack([np.tile(q_norm_w, 2), np.tile(q_norm_b, 2)], axis=1).astype(np.float32)
    kn_wb = np.stack([np.tile(k_norm_w, 2), np.tile(k_norm_b, 2)], axis=1).astype(np.float32)
    in_maps = []
    for c in range(N_CORES):
        b, s = c // 4, c % 4
        xt = np.ascontiguousarray(x[b, s * NLOC:(s + 1) * NLOC, :].T).astype(BF16)
        in_maps.append({
            "xT": xt, "wqkvT": wqkvT, "qkvb": qkvb, "wpT": wpT, "pb": pb,
            "qn_wb": qn_wb, "kn_wb": kn_wb,
        })
    return in_maps


def _install_ntff_hook_shim():
    """The agent image's antenv lacks axon_hooks; recreate it so trace=True
    can register the NTFF profile hook that trn_boot would have set."""
    import types
    import antenv

    if "antenv.axon_hooks" in sys.modules:
        return
    mod = types.ModuleType("antenv.axon_hooks")
    state = {"fn": None}
    mod.set_axon_ntff_profile_hook = lambda fn: state.__setitem__("fn", fn)
    mod.get_axon_ntff_profile_hook = lambda: state["fn"]
    sys.modules["antenv.axon_hooks"] = mod
    antenv.axon_hooks = mod
    try:
        from trn_agent_boot.trn_boot import _ntff_profile_via_ctypes
        hook = _ntff_profile_via_ctypes("/opt/axon/libaxon_pjrt.so")
        if hook is not None:
            mod.set_axon_ntff_profile_hook(hook)
    except Exception as e:  # degrade to no tracing
        print(f"ntff hook shim failed: {e}", file=sys.stderr)


def kernel(x, qkv_w, qkv_b, q_norm_w, q_norm_b, k_norm_w, k_norm_b,
           proj_w, proj_b, _trace=False):
    from concourse.bass_utils import run_bass_kernel_spmd

    if _trace:
        _install_ntff_hook_shim()

    if "nc" not in _COMPILED:
        _COMPILED["nc"] = build_graph()
    nc = _COMPILED["nc"]

    in_maps = _prep_in_maps(x, qkv_w, qkv_b, q_norm_w, q_norm_b,
                            k_norm_w, k_norm_b, proj_w, proj_b)
    res = run_bass_kernel_spmd(nc, in_maps, core_ids=list(range(N_CORES)),
                               trace=_trace)
    out = np.empty((B, N, C), dtype=np.float32)
    for c in range(N_CORES):
        b, s = c // 4, c % 4
        out[b, s * NLOC:(s + 1) * NLOC, :] = res.results[c]["out"]
    if _trace:
        _COMPILED["last_exec_time_ns"] = res.exec_time_ns
        _COMPILED["last_results"] = res
    return out
